# revision 11
# baseline (speedup 1.0000x reference)
"""CosformerAttention (causal linear attention) Trainium2 Bass kernel.

Full inputs in, full output out. Shards batch*heads over 8 NeuronCores:
device d handles sample n = d//4 and heads hA = 2*(d%4), hB = hA+1.
Each device computes q/k/v projections for its 2 heads, the chunked
causal linear attention scan, and a partial output projection over its
128 local features; the host sums the 4 per-sample partials.

Self-contained: hardcodes L=1024, N=2, E=512, H=8 from the problem spec.
"""

import sys

if "/opt/trn_rl_repo" not in sys.path:
    sys.path.insert(0, "/opt/trn_rl_repo")

import numpy as np

import concourse.bass as bass
import concourse.tile as tile
from concourse import mybir
import concourse.bass_utils as bass_utils
from concourse.vector_clock import ScopedClock

F32 = mybir.dt.float32
F32R = mybir.dt.float32r
ALU = mybir.AluOpType
ACTF = mybir.ActivationFunctionType

L, N, E, H = 1024, 2, 512, 8
D = E // H          # 64 head dim
DD = 2 * D          # 128 cos/sin-doubled head dim
P = 128             # partitions / chunk size
NCHUNK = L // P     # 8
NCORES = 8
EPS = 1e-6


# ---------------------------------------------------------------------------
# Tile tail-drain fix: this walrus build allows at most ONE semaphore wait
# per instruction, but TileContext._drain_and_barrier puts the whole global
# clock on a single Drain. Split the waits across preceding SP nops.
# ---------------------------------------------------------------------------
def _patched_drain_and_barrier(self, tick_clock, wait_clock):
    nc = self.nc
    nops = [nc.sync.nop() for _ in range(40)]
    drain_inst = nc.sync.drain()
    wait_clock.add_sem_waits(
        drain_inst.ins, ScopedClock({None: tick_clock.global_clock})
    )
    waits = list(drain_inst.ins.sync_info.on_wait or [])
    if len(waits) > 1:
        drain_inst.ins.sync_info.on_wait = [waits[-1]]
        SI = type(drain_inst.ins.sync_info)
        for nop, w in zip(nops, waits[:-1]):
            si = nop.ins.sync_info
            if si is None:
                nop.ins.sync_info = SI(on_wait=[w], on_update=[])
            else:
                si.on_wait = [w]
    nc.all_engine_barrier()
    popped = nc._tile_sem_poison_stack.pop()
    assert popped is self._sem_poison
    nc.clear_and_free_semaphores(list(self.sems.allocated().values()))
    nc.all_engine_barrier()


tile.TileContext._drain_and_barrier = _patched_drain_and_barrier


def r(ap):
    """Bitcast an fp32 AP to float32r for full-rate PE matmul."""
    return ap.bitcast(F32R)


def _split_multi_waits(nc, skip_opcodes=()):
    """Walrus codegen here allows at most one sem wait per engine
    instruction. Move excess waits onto preceding same-engine NoOps
    (engines execute strictly in order, so this is equivalent)."""
    k = 0
    for f in nc.m.functions:
        for bb in f.blocks:
            insts = list(bb.instructions)
            out, changed = [], False
            for inst in insts:
                si = inst.sync_info
                waits = list(si.on_wait) if (si is not None and si.on_wait) else []
                if (
                    len(waits) > 1
                    and type(inst).__name__ not in skip_opcodes
                    and "Unassigned" not in str(inst.engine)
                ):
                    for w in waits[:-1]:
                        nop = mybir.InstNoOp(name=f"wsplit-{k}", ins=[], outs=[])
                        k += 1
                        nop.engine = inst.engine
                        nop.sync_info = type(si)(on_wait=[w], on_update=[])
                        out.append(nop)
                    si.on_wait = [waits[-1]]
                    changed = True
                out.append(inst)
            if changed:
                bb.instructions = out


def build_program():
    nc = bass.Bass("TRN2", target_bir_lowering=False)

    # ---- DRAM I/O ----------------------------------------------------------
    xT_d = nc.dram_tensor("xT", [E, L], F32R, kind="ExternalInput").ap()
    wq_d = nc.dram_tensor("wq_f", [E, 2 * P], F32R, kind="ExternalInput").ap()
    wk_d = nc.dram_tensor("wk_f", [E, 2 * P], F32R, kind="ExternalInput").ap()
    wvk_d = nc.dram_tensor("w_vk", [E, 2 * P], F32R, kind="ExternalInput").ap()
    outw_d = nc.dram_tensor("outwT", [P, E], F32R, kind="ExternalInput").ap()
    sc_d = nc.dram_tensor("sc_full", [P, L], F32, kind="ExternalInput").ap()
    mask_d = nc.dram_tensor("mask", [P, P], F32, kind="ExternalInput").ap()
    id_d = nc.dram_tensor("ident", [P, P], F32, kind="ExternalInput").ap()
    scol_d = nc.dram_tensor("s_col", [P, NCHUNK], F32, kind="ExternalInput").ap()
    ccol_d = nc.dram_tensor("c_col", [P, NCHUNK], F32, kind="ExternalInput").ap()
    qb_d = nc.dram_tensor("qb_f", [P, 2], F32, kind="ExternalInput").ap()
    kb_d = nc.dram_tensor("kb_f", [P, 2], F32, kind="ExternalInput").ap()
    vkb_d = nc.dram_tensor("vkb", [1, 2 * P], F32R, kind="ExternalInput").ap()
    ones_d = nc.dram_tensor("ones_row", [1, P], F32R, kind="ExternalInput").ap()
    out_d = nc.dram_tensor("out", [L, E], F32, kind="ExternalOutput").ap()

    with tile.TileContext(nc) as tc:
        persist = tc.alloc_tile_pool(name="persist", bufs=1)
        work = tc.alloc_tile_pool(name="work", bufs=3)
        small = tc.alloc_tile_pool(name="small", bufs=4)
        ps_big = tc.alloc_tile_pool(name="ps_big", bufs=2, space="PSUM")
        ps_sq = tc.alloc_tile_pool(name="ps_sq", bufs=2, space="PSUM")
        ps_o = tc.alloc_tile_pool(name="ps_o", bufs=2, space="PSUM")
        ps_state = tc.alloc_tile_pool(name="ps_state", bufs=1, space="PSUM")

        # ---- constant / weight loads --------------------------------------
        def load(tag, shape, src, dt=F32):
            t = persist.tile(shape, dt, tag=tag, name=tag)
            nc.sync.dma_start(out=t[:], in_=src)
            return t

        xT = [load(f"x{e}", [P, L], xT_d[e * P:(e + 1) * P, :], F32R) for e in range(4)]
        wq = [load(f"wq{e}", [P, 2 * P], wq_d[e * P:(e + 1) * P, :], F32R) for e in range(4)]
        wk = [load(f"wk{e}", [P, 2 * P], wk_d[e * P:(e + 1) * P, :], F32R) for e in range(4)]
        wvk = [load(f"wvk{e}", [P, 2 * P], wvk_d[e * P:(e + 1) * P, :], F32R) for e in range(4)]
        outw = load("outw", [P, E], outw_d, F32R)
        sc = load("sc", [P, L], sc_d)
        mask = load("mask", [P, P], mask_d)
        ident = load("ident", [P, P], id_d)
        scol = load("scol", [P, NCHUNK], scol_d)
        ccol = load("ccol", [P, NCHUNK], ccol_d)
        qb = load("qb", [P, 2], qb_d)
        kb = load("kb", [P, 2], kb_d)
        vkb = load("vkb", [1, 2 * P], vkb_d, F32R)
        ones_row = load("ones", [1, P], ones_d, F32R)

        # persistent activations
        q_f = [persist.tile([P, L], F32, tag=f"qf{h}", name=f"qf{h}") for h in range(2)]
        k_f = [persist.tile([P, L], F32, tag=f"kf{h}", name=f"kf{h}") for h in range(2)]
        k_t = [persist.tile([P, NCHUNK, P], F32, tag=f"kt{h}", name=f"kt{h}") for h in range(2)]
        v_t = [persist.tile([P, NCHUNK, D + 1], F32, tag=f"vt{h}", name=f"vt{h}") for h in range(2)]
        attn = persist.tile([P, NCHUNK, P], F32, tag="attn")

        # ---- stage B: feature-layout q_/k_ ((2d, L), scaled by sin/cos) ----
        # stationary sets: 0=qA,1=qB,2=kA,3=kB; psum rows = head feats doubled
        for si in range(4):
            wf = wq[0] if si < 2 else wk[0]
            wlist = wq if si < 2 else wk
            bias = qb if si < 2 else kb
            h = si % 2
            dst = q_f[h] if si < 2 else k_f[h]
            for tch in range(2):
                ps = ps_big.tile([P, 512], F32, tag="big")
                for e in range(4):
                    nc.tensor.matmul(
                        ps[:],
                        wlist[e][:, h * P:(h + 1) * P],
                        xT[e][:, tch * 512:(tch + 1) * 512],
                        start=(e == 0),
                        stop=(e == 3),
                    )
                tmp = work.tile([P, 512], F32, tag="brelu")
                nc.scalar.activation(
                    tmp[:], ps[:], ACTF.Relu, bias=bias[:, h:h + 1], scale=1.0
                )
                nc.vector.tensor_mul(
                    dst[:, tch * 512:(tch + 1) * 512],
                    tmp[:],
                    sc[:, tch * 512:(tch + 1) * 512],
                )

        # ---- stage C: sequence-layout v (with ones col) and k_ -------------
        # psum cols: 0:64 vA, 64:128 vB, 128:192 kA, 192:256 kB
        for ch in range(NCHUNK):
            ps = ps_big.tile([P, 2 * P], F32, tag="big")
            nc.tensor.matmul(ps[:], ones_row[:], vkb[:], start=True, stop=False)
            for e in range(4):
                nc.tensor.matmul(
                    ps[:],
                    xT[e][:, ch * P:(ch + 1) * P],
                    wvk[e][:],
                    start=False,
                    stop=(e == 3),
                )
            for h in range(2):
                nc.vector.tensor_copy(v_t[h][:, ch, 0:D], ps[:, h * D:(h + 1) * D])
                nc.vector.memset(v_t[h][:, ch, D:D + 1], 1.0)
                kcols = ps[:, 2 * D + h * D: 2 * D + (h + 1) * D]
                nc.vector.tensor_scalar(
                    k_t[h][:, ch, 0:D], kcols, scalar1=0.0,
                    scalar2=scol[:, ch:ch + 1], op0=ALU.max, op1=ALU.mult,
                )
                nc.vector.tensor_scalar(
                    k_t[h][:, ch, D:DD], kcols, scalar1=0.0,
                    scalar2=ccol[:, ch:ch + 1], op0=ALU.max, op1=ALU.mult,
                )

        # ---- stage D: chunked causal linear attention ----------------------
        S = [ps_state.tile([P, D + 1], F32, tag=f"S{h}", name=f"S{h}") for h in range(2)]
        for ch in range(NCHUNK):
            cs = slice(ch * P, (ch + 1) * P)
            for h in range(2):
                if ch > 0:
                    S_sb = work.tile([P, D + 1], F32, tag="ssb")
                    nc.vector.tensor_copy(S_sb[:], S[h][:])
                # scoresT[l', l] for this chunk
                pss = ps_sq.tile([P, P], F32, tag="sq")
                nc.tensor.matmul(pss[:], k_f[h][:, cs], q_f[h][:, cs],
                                 start=True, stop=True)
                ms = work.tile([P, P], F32, tag="ms")
                nc.vector.tensor_mul(ms[:], pss[:], mask[:])
                # chunk output: intra + inter (both accumulate in psum)
                po = ps_o.tile([P, D + 1], F32, tag="po")
                nc.tensor.matmul(po[:], ms[:], v_t[h][:, ch, :],
                                 start=True, stop=(ch == 0))
                if ch > 0:
                    nc.tensor.matmul(po[:], q_f[h][:, cs], S_sb[:],
                                     start=False, stop=True)
                # state update AFTER the read of S for this chunk
                nc.tensor.matmul(S[h][:], k_t[h][:, ch, :], v_t[h][:, ch, :],
                                 start=(ch == 0), stop=(ch == NCHUNK - 1),
                                 skip_group_check=True)
                # attn = qkv / max(denom, eps)
                den = small.tile([P, 1], F32, tag="den")
                nc.vector.tensor_scalar_max(den[:], po[:, D:D + 1], EPS)
                rec = small.tile([P, 1], F32, tag="rec")
                nc.vector.reciprocal(rec[:], den[:])
                nc.vector.tensor_scalar_mul(
                    attn[:, ch, h * D:(h + 1) * D], po[:, 0:D], rec[:]
                )

        # ---- stage E: transpose + partial output projection ----------------
        for ch in range(NCHUNK):
            pst = ps_sq.tile([P, P], F32, tag="sq")
            nc.tensor.transpose(pst[:], attn[:, ch, :], ident[:])
            aT = work.tile([P, P], F32R, tag="aT")
            nc.vector.tensor_copy(aT[:], pst[:])
            pso = ps_big.tile([P, E], F32, tag="big")
            nc.tensor.matmul(pso[:], aT[:], outw[:], start=True, stop=True)
            osb = work.tile([P, E], F32, tag="osb")
            nc.vector.tensor_copy(osb[:], pso[:])
            nc.sync.dma_start(out=out_d[ch * P:(ch + 1) * P, :], in_=osb[:])

        for p in (ps_state, ps_o, ps_sq, ps_big, small, work, persist):
            p.release()

    _split_multi_waits(nc)
    return nc


_PROG = {}


def _get_program():
    if "nc" not in _PROG:
        _PROG["nc"] = build_program()
    return _PROG["nc"]


def _prep_core_inputs(dev, query, q_w, q_b, k_w, k_b, v_w, v_b, out_w):
    n = dev // 4
    hA = 2 * (dev % 4)
    a, b = hA * D, (hA + 1) * D

    def dup(w, lo):
        wt = w[lo:lo + D, :].T  # (E, 64)
        return np.concatenate([wt, wt], axis=1)  # (E, 128)

    xT = np.ascontiguousarray(query[:, n, :].T.astype(np.float32))
    wq_f = np.ascontiguousarray(np.concatenate([dup(q_w, a), dup(q_w, b)], axis=1))
    wk_f = np.ascontiguousarray(np.concatenate([dup(k_w, a), dup(k_w, b)], axis=1))
    w_vk = np.ascontiguousarray(np.concatenate(
        [v_w[a:a + D, :].T, v_w[b:b + D, :].T,
         k_w[a:a + D, :].T, k_w[b:b + D, :].T], axis=1))
    outwT = np.ascontiguousarray(
        np.concatenate([out_w[:, a:a + D].T, out_w[:, b:b + D].T], axis=0))

    idx = np.arange(1, L + 1, dtype=np.float64) * (np.pi / 2) / L
    s = np.sin(idx).astype(np.float32)
    c = np.cos(idx).astype(np.float32)
    sc_full = np.concatenate(
        [np.broadcast_to(s, (D, L)), np.broadcast_to(c, (D, L))], axis=0
    ).astype(np.float32)
    s_col = np.ascontiguousarray(s.reshape(NCHUNK, P).T)
    c_col = np.ascontiguousarray(c.reshape(NCHUNK, P).T)
    pi = np.arange(P)
    mask = (pi[:, None] <= pi[None, :]).astype(np.float32)
    ident = np.eye(P, dtype=np.float32)
    qb_f = np.stack(
        [np.concatenate([q_b[a:a + D]] * 2), np.concatenate([q_b[b:b + D]] * 2)],
        axis=1).astype(np.float32)
    kb_f = np.stack(
        [np.concatenate([k_b[a:a + D]] * 2), np.concatenate([k_b[b:b + D]] * 2)],
        axis=1).astype(np.float32)
    vkb = np.concatenate(
        [v_b[a:a + D], v_b[b:b + D], k_b[a:a + D], k_b[b:b + D]]
    ).reshape(1, 2 * P).astype(np.float32)

    return {
        "xT": xT, "wq_f": wq_f, "wk_f": wk_f, "w_vk": w_vk, "outwT": outwT,
        "sc_full": np.ascontiguousarray(sc_full), "mask": mask, "ident": ident,
        "s_col": s_col, "c_col": c_col, "qb_f": qb_f, "kb_f": kb_f, "vkb": vkb,
        "ones_row": np.ones((1, P), dtype=np.float32),
    }


def run(inputs, trace=False, trace_kwargs=None):
    nc = _get_program()
    in_maps = [
        _prep_core_inputs(
            d, inputs["query"], inputs["q_w"], inputs["q_b"], inputs["k_w"],
            inputs["k_b"], inputs["v_w"], inputs["v_b"], inputs["out_w"])
        for d in range(NCORES)
    ]
    res = bass_utils.run_bass_kernel_spmd(
        nc, in_maps, list(range(NCORES)), trace=trace,
        **(trace_kwargs or {}),
    )
    parts = [res.results[i]["out"] for i in range(NCORES)]
    out0 = parts[0] + parts[1] + parts[2] + parts[3]
    out1 = parts[4] + parts[5] + parts[6] + parts[7]
    out = np.stack([out0, out1], axis=1) + inputs["out_b"][None, None, :]
    return out.astype(np.float32), res


def kernel(**inputs) -> np.ndarray:
    out, _ = run(inputs, trace=False)
    return out


# revision 14
# speedup vs baseline: 1.2701x; 1.2701x over previous
"""CosformerAttention (causal linear attention) Trainium2 Bass kernel.

Full inputs in, full output out. Shards batch*heads over 8 NeuronCores:
device d handles sample n = d//4 and heads hA = 2*(d%4), hB = hA+1.
Each device computes q/k/v projections for its 2 heads, the chunked
causal linear attention scan, and a partial output projection over its
128 local features; the host sums the 4 per-sample partials.

Self-contained: hardcodes L=1024, N=2, E=512, H=8 from the problem spec.
"""

import sys

if "/opt/trn_rl_repo" not in sys.path:
    sys.path.insert(0, "/opt/trn_rl_repo")

import numpy as np
import ml_dtypes

BF16NP = ml_dtypes.bfloat16

import concourse.bass as bass
import concourse.tile as tile
from concourse import mybir
import concourse.bass_utils as bass_utils
from concourse.vector_clock import ScopedClock

F32 = mybir.dt.float32
F32R = mybir.dt.float32r
BF16 = mybir.dt.bfloat16
ALU = mybir.AluOpType
ACTF = mybir.ActivationFunctionType

L, N, E, H = 1024, 2, 512, 8
D = E // H          # 64 head dim
DD = 2 * D          # 128 cos/sin-doubled head dim
P = 128             # partitions / chunk size
NCHUNK = L // P     # 8
NCORES = 8
EPS = 1e-6


# ---------------------------------------------------------------------------
# Tile tail-drain fix: this walrus build allows at most ONE semaphore wait
# per instruction, but TileContext._drain_and_barrier puts the whole global
# clock on a single Drain. Split the waits across preceding SP nops.
# ---------------------------------------------------------------------------
def _patched_drain_and_barrier(self, tick_clock, wait_clock):
    nc = self.nc
    nops = [nc.sync.nop() for _ in range(40)]
    drain_inst = nc.sync.drain()
    wait_clock.add_sem_waits(
        drain_inst.ins, ScopedClock({None: tick_clock.global_clock})
    )
    waits = list(drain_inst.ins.sync_info.on_wait or [])
    if len(waits) > 1:
        drain_inst.ins.sync_info.on_wait = [waits[-1]]
        SI = type(drain_inst.ins.sync_info)
        for nop, w in zip(nops, waits[:-1]):
            si = nop.ins.sync_info
            if si is None:
                nop.ins.sync_info = SI(on_wait=[w], on_update=[])
            else:
                si.on_wait = [w]
    nc.all_engine_barrier()
    popped = nc._tile_sem_poison_stack.pop()
    assert popped is self._sem_poison
    nc.clear_and_free_semaphores(list(self.sems.allocated().values()))
    nc.all_engine_barrier()


tile.TileContext._drain_and_barrier = _patched_drain_and_barrier


def r(ap):
    """Bitcast an fp32 AP to float32r for full-rate PE matmul."""
    return ap.bitcast(F32R)


def _split_multi_waits(nc, skip_opcodes=()):
    """Walrus codegen here allows at most one sem wait per engine
    instruction. Move excess waits onto preceding same-engine NoOps
    (engines execute strictly in order, so this is equivalent)."""
    k = 0
    for f in nc.m.functions:
        for bb in f.blocks:
            insts = list(bb.instructions)
            out, changed = [], False
            for inst in insts:
                si = inst.sync_info
                waits = list(si.on_wait) if (si is not None and si.on_wait) else []
                if (
                    len(waits) > 1
                    and type(inst).__name__ not in skip_opcodes
                    and "Unassigned" not in str(inst.engine)
                ):
                    for w in waits[:-1]:
                        nop = mybir.InstNoOp(name=f"wsplit-{k}", ins=[], outs=[])
                        k += 1
                        nop.engine = inst.engine
                        nop.sync_info = type(si)(on_wait=[w], on_update=[])
                        out.append(nop)
                    si.on_wait = [waits[-1]]
                    changed = True
                out.append(inst)
            if changed:
                bb.instructions = out


def build_program():
    nc = bass.Bass("TRN2", target_bir_lowering=False)

    # ---- DRAM I/O ----------------------------------------------------------
    xT_d = nc.dram_tensor("xT", [E, L], BF16, kind="ExternalInput").ap()
    wq_d = nc.dram_tensor("wq_f", [E, 2 * P], BF16, kind="ExternalInput").ap()
    wk_d = nc.dram_tensor("wk_f", [E, 2 * P], BF16, kind="ExternalInput").ap()
    wvk_d = nc.dram_tensor("w_vk", [E, 2 * P], BF16, kind="ExternalInput").ap()
    outw_d = nc.dram_tensor("outwT", [P, E], BF16, kind="ExternalInput").ap()
    sc_d = nc.dram_tensor("sc_full", [P, L], F32, kind="ExternalInput").ap()
    mask_d = nc.dram_tensor("mask", [P, P], F32, kind="ExternalInput").ap()
    id_d = nc.dram_tensor("ident", [P, P], BF16, kind="ExternalInput").ap()
    scol_d = nc.dram_tensor("s_col", [P, NCHUNK], F32, kind="ExternalInput").ap()
    ccol_d = nc.dram_tensor("c_col", [P, NCHUNK], F32, kind="ExternalInput").ap()
    qb_d = nc.dram_tensor("qb_f", [P, 2], F32, kind="ExternalInput").ap()
    kb_d = nc.dram_tensor("kb_f", [P, 2], F32, kind="ExternalInput").ap()
    vkb_d = nc.dram_tensor("vkb", [1, 2 * P], BF16, kind="ExternalInput").ap()
    ones_d = nc.dram_tensor("ones_row", [1, P], BF16, kind="ExternalInput").ap()
    out_d = nc.dram_tensor("out", [L, E], F32, kind="ExternalOutput").ap()

    with tile.TileContext(nc) as tc:
        persist = tc.alloc_tile_pool(name="persist", bufs=1)
        work = tc.alloc_tile_pool(name="work", bufs=3)
        small = tc.alloc_tile_pool(name="small", bufs=4)
        ps_big = tc.alloc_tile_pool(name="ps_big", bufs=2, space="PSUM")
        ps_sq = tc.alloc_tile_pool(name="ps_sq", bufs=2, space="PSUM")
        ps_o = tc.alloc_tile_pool(name="ps_o", bufs=2, space="PSUM")
        ps_state = tc.alloc_tile_pool(name="ps_state", bufs=1, space="PSUM")

        # ---- constant / weight loads --------------------------------------
        def load(tag, shape, src, dt=F32):
            t = persist.tile(shape, dt, tag=tag, name=tag)
            nc.sync.dma_start(out=t[:], in_=src)
            return t

        xT = [load(f"x{e}", [P, L], xT_d[e * P:(e + 1) * P, :], BF16) for e in range(4)]
        wq = [load(f"wq{e}", [P, 2 * P], wq_d[e * P:(e + 1) * P, :], BF16) for e in range(4)]
        wk = [load(f"wk{e}", [P, 2 * P], wk_d[e * P:(e + 1) * P, :], BF16) for e in range(4)]
        wvk = [load(f"wvk{e}", [P, 2 * P], wvk_d[e * P:(e + 1) * P, :], BF16) for e in range(4)]
        outw = load("outw", [P, E], outw_d, BF16)
        sc = load("sc", [P, L], sc_d)
        mask = load("mask", [P, P], mask_d)
        ident = load("ident", [P, P], id_d, BF16)
        scol = load("scol", [P, NCHUNK], scol_d)
        ccol = load("ccol", [P, NCHUNK], ccol_d)
        qb = load("qb", [P, 2], qb_d)
        kb = load("kb", [P, 2], kb_d)
        vkb = load("vkb", [1, 2 * P], vkb_d, BF16)
        ones_row = load("ones", [1, P], ones_d, BF16)

        # persistent activations
        q_f = [persist.tile([P, L], BF16, tag=f"qf{h}", name=f"qf{h}") for h in range(2)]
        k_f = [persist.tile([P, L], BF16, tag=f"kf{h}", name=f"kf{h}") for h in range(2)]
        k_t = [persist.tile([P, NCHUNK, P], BF16, tag=f"kt{h}", name=f"kt{h}") for h in range(2)]
        v_t = [persist.tile([P, NCHUNK, D + 1], BF16, tag=f"vt{h}", name=f"vt{h}") for h in range(2)]
        attn = persist.tile([P, NCHUNK, P], BF16, tag="attn")

        # ---- stage B: feature-layout q_/k_ ((2d, L), scaled by sin/cos) ----
        # stationary sets: 0=qA,1=qB,2=kA,3=kB; psum rows = head feats doubled
        for si in range(4):
            wf = wq[0] if si < 2 else wk[0]
            wlist = wq if si < 2 else wk
            bias = qb if si < 2 else kb
            h = si % 2
            dst = q_f[h] if si < 2 else k_f[h]
            for tch in range(2):
                ps = ps_big.tile([P, 512], F32, tag="big")
                for e in range(4):
                    nc.tensor.matmul(
                        ps[:],
                        wlist[e][:, h * P:(h + 1) * P],
                        xT[e][:, tch * 512:(tch + 1) * 512],
                        start=(e == 0),
                        stop=(e == 3),
                    )
                tmp = work.tile([P, 512], F32, tag="brelu")
                nc.scalar.activation(
                    tmp[:], ps[:], ACTF.Relu, bias=bias[:, h:h + 1], scale=1.0
                )
                nc.vector.tensor_mul(
                    dst[:, tch * 512:(tch + 1) * 512],
                    tmp[:],
                    sc[:, tch * 512:(tch + 1) * 512],
                )

        # ---- stage C: sequence-layout v (with ones col) and k_ -------------
        # psum cols: 0:64 vA, 64:128 vB, 128:192 kA, 192:256 kB
        for ch in range(NCHUNK):
            ps = ps_big.tile([P, 2 * P], F32, tag="big")
            nc.tensor.matmul(ps[:], ones_row[:], vkb[:], start=True, stop=False)
            for e in range(4):
                nc.tensor.matmul(
                    ps[:],
                    xT[e][:, ch * P:(ch + 1) * P],
                    wvk[e][:],
                    start=False,
                    stop=(e == 3),
                )
            for h in range(2):
                nc.vector.tensor_copy(v_t[h][:, ch, 0:D], ps[:, h * D:(h + 1) * D])
                nc.vector.memset(v_t[h][:, ch, D:D + 1], 1.0)
                kcols = ps[:, 2 * D + h * D: 2 * D + (h + 1) * D]
                nc.vector.tensor_scalar(
                    k_t[h][:, ch, 0:D], kcols, scalar1=0.0,
                    scalar2=scol[:, ch:ch + 1], op0=ALU.max, op1=ALU.mult,
                )
                nc.vector.tensor_scalar(
                    k_t[h][:, ch, D:DD], kcols, scalar1=0.0,
                    scalar2=ccol[:, ch:ch + 1], op0=ALU.max, op1=ALU.mult,
                )

        # ---- stage D: chunked causal linear attention ----------------------
        S = [ps_state.tile([P, D + 1], F32, tag=f"S{h}", name=f"S{h}") for h in range(2)]
        for ch in range(NCHUNK):
            cs = slice(ch * P, (ch + 1) * P)
            for h in range(2):
                if ch > 0:
                    S_sb = work.tile([P, D + 1], BF16, tag="ssb")
                    nc.vector.tensor_copy(S_sb[:], S[h][:])
                # scoresT[l', l] for this chunk
                pss = ps_sq.tile([P, P], F32, tag="sq")
                nc.tensor.matmul(pss[:], k_f[h][:, cs], q_f[h][:, cs],
                                 start=True, stop=True)
                ms = work.tile([P, P], BF16, tag="ms")
                nc.vector.tensor_mul(ms[:], pss[:], mask[:])
                # chunk output: intra + inter (both accumulate in psum)
                po = ps_o.tile([P, D + 1], F32, tag="po")
                nc.tensor.matmul(po[:], ms[:], v_t[h][:, ch, :],
                                 start=True, stop=(ch == 0))
                if ch > 0:
                    nc.tensor.matmul(po[:], q_f[h][:, cs], S_sb[:],
                                     start=False, stop=True)
                # state update AFTER the read of S for this chunk
                nc.tensor.matmul(S[h][:], k_t[h][:, ch, :], v_t[h][:, ch, :],
                                 start=(ch == 0), stop=(ch == NCHUNK - 1),
                                 skip_group_check=True)
                # attn = qkv / max(denom, eps)
                den = small.tile([P, 1], F32, tag="den")
                nc.vector.tensor_scalar_max(den[:], po[:, D:D + 1], EPS)
                rec = small.tile([P, 1], F32, tag="rec")
                nc.vector.reciprocal(rec[:], den[:])
                nc.vector.tensor_scalar_mul(
                    attn[:, ch, h * D:(h + 1) * D], po[:, 0:D], rec[:]
                )

        # ---- stage E: transpose + partial output projection ----------------
        for ch in range(NCHUNK):
            pst = ps_sq.tile([P, P], BF16, tag="sq")
            nc.tensor.transpose(pst[:], attn[:, ch, :], ident[:])
            aT = work.tile([P, P], BF16, tag="aT")
            nc.vector.tensor_copy(aT[:], pst[:])
            pso = ps_big.tile([P, E], F32, tag="big")
            nc.tensor.matmul(pso[:], aT[:], outw[:], start=True, stop=True)
            osb = work.tile([P, E], F32, tag="osb")
            nc.vector.tensor_copy(osb[:], pso[:])
            nc.sync.dma_start(out=out_d[ch * P:(ch + 1) * P, :], in_=osb[:])

        for p in (ps_state, ps_o, ps_sq, ps_big, small, work, persist):
            p.release()

    _split_multi_waits(nc)
    return nc


_PROG = {}


def _get_program():
    if "nc" not in _PROG:
        _PROG["nc"] = build_program()
    return _PROG["nc"]


def _prep_core_inputs(dev, query, q_w, q_b, k_w, k_b, v_w, v_b, out_w):
    n = dev // 4
    hA = 2 * (dev % 4)
    a, b = hA * D, (hA + 1) * D

    def dup(w, lo):
        wt = w[lo:lo + D, :].T  # (E, 64)
        return np.concatenate([wt, wt], axis=1)  # (E, 128)

    xT = np.ascontiguousarray(query[:, n, :].T.astype(np.float32))
    wq_f = np.ascontiguousarray(np.concatenate([dup(q_w, a), dup(q_w, b)], axis=1))
    wk_f = np.ascontiguousarray(np.concatenate([dup(k_w, a), dup(k_w, b)], axis=1))
    w_vk = np.ascontiguousarray(np.concatenate(
        [v_w[a:a + D, :].T, v_w[b:b + D, :].T,
         k_w[a:a + D, :].T, k_w[b:b + D, :].T], axis=1))
    outwT = np.ascontiguousarray(
        np.concatenate([out_w[:, a:a + D].T, out_w[:, b:b + D].T], axis=0))

    idx = np.arange(1, L + 1, dtype=np.float64) * (np.pi / 2) / L
    s = np.sin(idx).astype(np.float32)
    c = np.cos(idx).astype(np.float32)
    sc_full = np.concatenate(
        [np.broadcast_to(s, (D, L)), np.broadcast_to(c, (D, L))], axis=0
    ).astype(np.float32)
    s_col = np.ascontiguousarray(s.reshape(NCHUNK, P).T)
    c_col = np.ascontiguousarray(c.reshape(NCHUNK, P).T)
    pi = np.arange(P)
    mask = (pi[:, None] <= pi[None, :]).astype(np.float32)
    ident = np.eye(P, dtype=np.float32)
    qb_f = np.stack(
        [np.concatenate([q_b[a:a + D]] * 2), np.concatenate([q_b[b:b + D]] * 2)],
        axis=1).astype(np.float32)
    kb_f = np.stack(
        [np.concatenate([k_b[a:a + D]] * 2), np.concatenate([k_b[b:b + D]] * 2)],
        axis=1).astype(np.float32)
    vkb = np.concatenate(
        [v_b[a:a + D], v_b[b:b + D], k_b[a:a + D], k_b[b:b + D]]
    ).reshape(1, 2 * P).astype(np.float32)

    return {
        "xT": xT.astype(BF16NP), "wq_f": wq_f.astype(BF16NP),
        "wk_f": wk_f.astype(BF16NP), "w_vk": w_vk.astype(BF16NP),
        "outwT": outwT.astype(BF16NP),
        "sc_full": np.ascontiguousarray(sc_full), "mask": mask,
        "ident": ident.astype(BF16NP),
        "s_col": s_col, "c_col": c_col, "qb_f": qb_f, "kb_f": kb_f,
        "vkb": vkb.astype(BF16NP),
        "ones_row": np.ones((1, P), dtype=BF16NP),
    }


def run(inputs, trace=False, trace_kwargs=None):
    nc = _get_program()
    in_maps = [
        _prep_core_inputs(
            d, inputs["query"], inputs["q_w"], inputs["q_b"], inputs["k_w"],
            inputs["k_b"], inputs["v_w"], inputs["v_b"], inputs["out_w"])
        for d in range(NCORES)
    ]
    res = bass_utils.run_bass_kernel_spmd(
        nc, in_maps, list(range(NCORES)), trace=trace,
        **(trace_kwargs or {}),
    )
    parts = [res.results[i]["out"] for i in range(NCORES)]
    out0 = parts[0] + parts[1] + parts[2] + parts[3]
    out1 = parts[4] + parts[5] + parts[6] + parts[7]
    out = np.stack([out0, out1], axis=1) + inputs["out_b"][None, None, :]
    return out.astype(np.float32), res


def kernel(**inputs) -> np.ndarray:
    out, _ = run(inputs, trace=False)
    return out
